# revision 1
# baseline (speedup 1.0000x reference)
"""Group-wise correlation cost volume (build_gwc_volume) on 8 trn2 cores.

volume[b,g,d,h,w] = sum_c ref[b,g,c,h,w] * tgt[b,g,c,h,w-d]  (0 where w<d)

Sharding: 16 (b,g) pairs across 8 cores, 2 pairs per core. Each pair is a
contiguous 64-channel slice of the inputs and a contiguous [D,H,W] slab of
the output.

Per (b,g,h) the volume rows are diagonals of the Gram matrix
G[w',w] = sum_c tgt[c,w'] * ref[c,w].  The PE computes three band tiles:
  A = G[0:128, 0:128]      (band d = w - w' for w in [0,128))
  C = G[0:128, 128:176]    (rows 81:128 hold the band tail for w in [128,176))
  B = G[128:256, 128:256]  (band for w in [128,256))
Diagonal (shear) extraction is not AP-expressible on-chip, so the band
tiles are DMAed out and the diagonals are gathered on the host during
unsharding.
"""

import sys

if "/opt/trn_rl_repo" not in sys.path:
    sys.path.insert(0, "/opt/trn_rl_repo")

import numpy as np

import concourse.bacc as bacc
import concourse.tile as tile
from concourse import mybir
from concourse.bass_utils import run_bass_kernel_spmd

F32 = mybir.dt.float32

B, C, H, W = 2, 512, 128, 256
G, CG, D = 8, 64, 48
N_CORES = 8
PAIRS = 2  # (b,g) pairs per core
HC = 8  # h rows per chunk
CW = 47 + 1  # corner tile width (w in [128, 176))

_cached = {}


def _build_module():
    nc = bacc.Bacc("TRN2", target_bir_lowering=False, debug=False, num_devices=N_CORES)
    ref = nc.dram_tensor("ref", [PAIRS, CG, H, W], F32, kind="ExternalInput")
    tgt = nc.dram_tensor("tgt", [PAIRS, CG, H, W], F32, kind="ExternalInput")
    # band tiles, layout [pair, w', h, w]
    out_a = nc.dram_tensor("out_a", [PAIRS, 128, H, 128], F32, kind="ExternalOutput")
    out_b = nc.dram_tensor("out_b", [PAIRS, 128, H, 128], F32, kind="ExternalOutput")
    out_c = nc.dram_tensor("out_c", [PAIRS, 47, H, CW], F32, kind="ExternalOutput")

    with tile.TileContext(nc) as tc:
        with (
            tc.tile_pool(name="ins", bufs=3) as ins,
            tc.tile_pool(name="stage", bufs=3) as stage_pool,
            tc.tile_pool(name="psum", bufs=8, space="PSUM") as psum,
        ):
            for pr in range(PAIRS):
                for ch in range(H // HC):
                    h0 = ch * HC
                    rt = ins.tile([CG, HC, W], F32, tag="rt")
                    tt = ins.tile([CG, HC, W], F32, tag="tt")
                    nc.sync.dma_start(rt[:], ref[pr, :, h0 : h0 + HC, :])
                    nc.sync.dma_start(tt[:], tgt[pr, :, h0 : h0 + HC, :])
                    stage = stage_pool.tile([128, HC, 304], F32, tag="st")
                    for hl in range(HC):
                        bank = psum.tile([128, 512], F32, tag="bank")
                        # A: stationary T[:, 0:128]
                        nc.tensor.matmul(
                            bank[:, 0:128], tt[:, hl, 0:128], rt[:, hl, 0:128]
                        )
                        # C: same stationary, moving R[:, 128:176]
                        nc.tensor.matmul(
                            bank[:, 128 : 128 + CW],
                            tt[:, hl, 0:128],
                            rt[:, hl, 128 : 128 + CW],
                        )
                        # B: stationary T[:, 128:256]
                        nc.tensor.matmul(
                            bank[:, 176:304], tt[:, hl, 128:256], rt[:, hl, 128:256]
                        )
                        if hl % 2 == 0:
                            nc.vector.tensor_copy(stage[:, hl, :], bank[:, 0:304])
                        else:
                            nc.scalar.copy(stage[:, hl, :], bank[:, 0:304])
                    nc.sync.dma_start(
                        out_a[pr, :, h0 : h0 + HC, :], stage[:, :, 0:128]
                    )
                    nc.sync.dma_start(
                        out_c[pr, :, h0 : h0 + HC, :], stage[81:128, :, 128 : 128 + CW]
                    )
                    nc.sync.dma_start(
                        out_b[pr, :, h0 : h0 + HC, :], stage[:, :, 176:304]
                    )

    nc.compile()
    return nc


def _get_module():
    if "nc" not in _cached:
        _cached["nc"] = _build_module()
    return _cached["nc"]


def _host_extract(res_a, res_b, res_c):
    """Gather band-tile diagonals into the full volume.

    res_a/res_b: [16, 128, H, 128] (pair-major), res_c: [16, 47, H, CW].
    Returns volume [B, G, D, H, W].
    """
    d_idx = np.arange(D)[:, None]  # [D, 1]
    w_idx = np.arange(128)[None, :]  # [1, 128]
    wp = w_idx - d_idx  # [D, 128] source row in A/B
    wp_ok = wp >= 0
    wp_cl = np.clip(wp, 0, 127)
    # corner: row = wr + 47 - d in [0,47), valid when d > wr (wr = w - 128)
    wrc = np.arange(CW)[None, :]
    cp = wrc + 47 - d_idx
    use_c = d_idx > wrc  # [D, CW]
    cp_cl = np.clip(cp, 0, 46)

    vol = np.empty((B * G, D, H, W), np.float32)
    for pair in range(B * G):
        a = res_a[pair].transpose(1, 0, 2)  # [h, w', w]
        b = res_b[pair].transpose(1, 0, 2)
        c = res_c[pair].transpose(1, 0, 2)  # [h, 47, CW]
        # region 1: w in [0,128)
        r1 = a[:, wp_cl, w_idx]  # [h, D, 128]
        r1 *= wp_ok[None]
        vol[pair, :, :, 0:128] = r1.transpose(1, 0, 2)
        # region 2: w in [128,256) from B, corner part from C
        r2 = b[:, wp_cl, w_idx]  # [h, D, 128]
        rc = c[:, cp_cl, wrc]  # [h, D, CW]
        r2[:, :, :CW] = np.where(use_c[None], rc, r2[:, :, :CW])
        vol[pair, :, :, 128:256] = r2.transpose(1, 0, 2)
    return vol.reshape(B, G, D, H, W)


def kernel(refimg_fea, targetimg_fea, num_groups, maxdisp):
    assert int(num_groups) == G and int(maxdisp) == D
    ref = np.ascontiguousarray(refimg_fea, dtype=np.float32)
    tgt = np.ascontiguousarray(targetimg_fea, dtype=np.float32)
    assert ref.shape == (B, C, H, W)

    # core k handles (b,g) pairs 2k, 2k+1 -> channels [128k', ...) of batch b
    rp = ref.reshape(B * G, CG, H, W)
    tp = tgt.reshape(B * G, CG, H, W)
    in_maps = [
        {
            "ref": rp[2 * k : 2 * k + 2],
            "tgt": tp[2 * k : 2 * k + 2],
        }
        for k in range(N_CORES)
    ]

    nc = _get_module()
    res = run_bass_kernel_spmd(nc, in_maps, core_ids=list(range(N_CORES)))

    res_a = np.concatenate([r["out_a"] for r in res.results], axis=0)
    res_b = np.concatenate([r["out_b"] for r in res.results], axis=0)
    res_c = np.concatenate([r["out_c"] for r in res.results], axis=0)
    return _host_extract(res_a, res_b, res_c)


# revision 6
# speedup vs baseline: 2.5225x; 2.5225x over previous
"""Group-wise correlation cost volume (build_gwc_volume) on 8 trn2 cores.

volume[b,g,d,h,w] = sum_c ref[b,g,c,h,w] * tgt[b,g,c,h,w-d]  (0 where w<d)

Sharding: 16 (b,g) pairs across 8 cores, 2 pairs per core. Each pair is a
contiguous 64-channel slice of the inputs and a contiguous [D,H,W] slab of
the output.

Per (b,g,h) the volume rows are diagonals of the Gram matrix
G[w',w] = sum_c tgt[c,w'] * ref[c,w].  The PE computes two band tiles:
  AC = G[0:128, 0:176]     (band d = w - w' for w in [0,176))
  B  = G[128:256, 128:256] (band for w in [128,256))
The two (b,g) pairs sit on partition halves (pr*64 + c), so their K=64
matmuls occupy different PE row groups and run concurrently.

Diagonal (shear) extraction is not AP-expressible on-chip, so the band
tiles are DMAed out and the diagonals are gathered on the host during
unsharding.
"""

import sys

if "/opt/trn_rl_repo" not in sys.path:
    sys.path.insert(0, "/opt/trn_rl_repo")

import numpy as np

import concourse.bacc as bacc
import concourse.tile as tile
from concourse import mybir
from concourse.bass_utils import run_bass_kernel_spmd

F32 = mybir.dt.float32

B, C, H, W = 2, 512, 128, 256
G, CG, D = 8, 64, 48
N_CORES = 8
PAIRS = 2  # (b,g) pairs per core
HC = 16  # h rows per chunk
ACW = 176  # A|C tile width (w in [0,176))

_cached = {}


def _build_module():
    nc = bacc.Bacc("TRN2", target_bir_lowering=False, debug=False, num_devices=N_CORES)
    ref = nc.dram_tensor("ref", [PAIRS, CG, H, W], F32, kind="ExternalInput")
    tgt = nc.dram_tensor("tgt", [PAIRS, CG, H, W], F32, kind="ExternalInput")
    # band tiles, layout [pair, w', h, w] with contiguous (h, w) per row
    out_ac = nc.dram_tensor("out_ac", [PAIRS, 128, H, ACW], F32, kind="ExternalOutput")
    out_b = nc.dram_tensor("out_b", [PAIRS, 128, H, 128], F32, kind="ExternalOutput")

    ref_p = ref.rearrange("pr c h w -> (pr c) h w")
    tgt_p = tgt.rearrange("pr c h w -> (pr c) h w")

    with tile.TileContext(nc) as tc:
        with (
            tc.tile_pool(name="ins", bufs=2) as ins,
            tc.tile_pool(name="stage", bufs=2) as stage_pool,
            tc.tile_pool(name="psum", bufs=4, space="PSUM") as psum,
        ):
            for ch in range(H // HC):
                h0 = ch * HC
                rt = ins.tile([128, HC, W], F32, tag="rt")
                tt = ins.tile([128, HC, W], F32, tag="tt")
                nc.sync.dma_start(rt[:], ref_p[:, h0 : h0 + HC, :])
                nc.sync.dma_start(tt[:], tgt_p[:, h0 : h0 + HC, :])
                stages = []
                for pr in range(PAIRS):
                    sac_t = stage_pool.tile(
                        [128, HC, ACW], F32, tag=f"sac{pr}", name=f"sac{pr}_{ch}"
                    )
                    sb_t = stage_pool.tile(
                        [128, HC, 128], F32, tag=f"sb{pr}", name=f"sb{pr}_{ch}"
                    )
                    stages.append((sac_t, sb_t))
                for hl in range(HC):
                    bank_a = psum.tile([128, 512], F32, tag="bank_a")
                    bank_b = psum.tile([128, 512], F32, tag="bank_b")
                    banks = [bank_a, bank_b]
                    for pr in range(PAIRS):
                        p0 = pr * CG
                        bank = banks[pr]
                        # A|C: stationary T[:, 0:128], moving R[:, 0:176]
                        nc.tensor.matmul(
                            bank[:, 0:ACW],
                            tt[p0 : p0 + CG, hl, 0:128],
                            rt[p0 : p0 + CG, hl, 0:ACW],
                        )
                        # B: stationary T[:, 128:256], moving R[:, 128:256]
                        nc.tensor.matmul(
                            bank[:, ACW : ACW + 128],
                            tt[p0 : p0 + CG, hl, 128:256],
                            rt[p0 : p0 + CG, hl, 128:256],
                        )
                        sac, sb = stages[pr]
                        eng = nc.vector if (hl + pr) % 2 == 0 else nc.scalar
                        copy = (
                            eng.tensor_copy if eng is nc.vector else eng.copy
                        )
                        copy(sac[:, hl, :], bank[:, 0:ACW])
                        copy(sb[:, hl, :], bank[:, ACW : ACW + 128])
                for pr in range(PAIRS):
                    sac, sb = stages[pr]
                    nc.sync.dma_start(out_ac[pr, :, h0 : h0 + HC, :], sac[:])
                    nc.sync.dma_start(out_b[pr, :, h0 : h0 + HC, :], sb[:])

    nc.compile()
    return nc


def _get_module():
    if "nc" not in _cached:
        _cached["nc"] = _build_module()
    return _cached["nc"]


def _host_extract(res_ac, res_b):
    """Gather band-tile diagonals into the full volume.

    res_ac: [16, 128, H, 176], res_b: [16, 128, H, 128] (pair-major).
    Returns volume [B, G, D, H, W].
    """
    d_idx = np.arange(D)[:, None]  # [D, 1]
    w_idx = np.arange(128)[None, :]  # [1, 128]
    wp = w_idx - d_idx  # [D, 128] source row (w' = w - d)
    wp_ok = wp >= 0
    wp_cl = np.clip(wp, 0, 127)
    # region 2 corner (w in [128,176), w' in [81,128)): from AC cols 128:176
    wrc = np.arange(48)[None, :]
    cp = 128 + wrc - d_idx  # source row in AC
    use_c = d_idx > wrc
    cp_cl = np.clip(cp, 0, 127)

    vol = np.empty((B * G, D, H, W), np.float32)
    for pair in range(B * G):
        ac = res_ac[pair].transpose(1, 0, 2)  # [h, w', 176]
        b = res_b[pair].transpose(1, 0, 2)  # [h, w', 128]
        # region 1: w in [0,128)
        r1 = ac[:, wp_cl, w_idx]  # [h, D, 128]
        r1 *= wp_ok[None]
        vol[pair, :, :, 0:128] = r1.transpose(1, 0, 2)
        # region 2: w in [128,256): B tile rows wr-d, corner rows from AC
        r2 = b[:, wp_cl, w_idx]  # [h, D, 128]
        rc = ac[:, cp_cl, 128 + wrc]  # [h, D, 48]
        r2[:, :, :48] = np.where(use_c[None], rc, r2[:, :, :48])
        vol[pair, :, :, 128:256] = r2.transpose(1, 0, 2)
    return vol.reshape(B, G, D, H, W)


def kernel(refimg_fea, targetimg_fea, num_groups, maxdisp):
    assert int(num_groups) == G and int(maxdisp) == D
    ref = np.ascontiguousarray(refimg_fea, dtype=np.float32)
    tgt = np.ascontiguousarray(targetimg_fea, dtype=np.float32)
    assert ref.shape == (B, C, H, W)

    # core k handles (b,g) pairs 2k, 2k+1 -> channels [128k', ...) of batch b
    rp = ref.reshape(B * G, CG, H, W)
    tp = tgt.reshape(B * G, CG, H, W)
    in_maps = [
        {
            "ref": rp[2 * k : 2 * k + 2],
            "tgt": tp[2 * k : 2 * k + 2],
        }
        for k in range(N_CORES)
    ]

    nc = _get_module()
    res = run_bass_kernel_spmd(nc, in_maps, core_ids=list(range(N_CORES)))

    res_ac = np.concatenate([r["out_ac"] for r in res.results], axis=0)
    res_b = np.concatenate([r["out_b"] for r in res.results], axis=0)
    return _host_extract(res_ac, res_b)
